# revision 24
# baseline (speedup 1.0000x reference)
"""Distributed flash-attention Bass kernel for 8 TRN2 NeuronCores.

Problem: nn_Attention (B=2, N=4096, C=512, H=8 heads, hd=64), f32 I/O.

Sharding: batch x head-pair. Core c handles batch c//4, heads {2*(c%4),
2*(c%4)+1}, over the FULL 4096-query x 4096-key attention. Each core
projects q/k/v for only its two heads (w_qkv column slice -> no
duplicated K/V compute), runs no-max flash attention, and applies its
w_proj slice to produce a PARTIAL [4096, 512] f32 output. The host sums
the 4 partials per batch and adds b_proj (numpy; no device collectives).

Engine plan (measured steady-state rates):
  PE   per step (128 keys x 2 heads x 512 queries): S-pair = two
       row-tiled K=64 matmuls running concurrently in PE row groups
       (~385ns), then two PV K=128 matmuls (~215ns each, drains
       overlapped). Fills/norm-broadcast/proj matmuls are paced into
       the stream. PV trails S/exp by TWO steps so the in-order PE
       queue never waits on the exp latency.
  Act  exp of 4/7 of the [128,1024] score tiles (exact table exp,
       whole tiles to amortize overhead) + PSUM evictions.
  DVE  exp of 3/7 of tiles via the Schraudolph bit-trick: a single
       tensor_scalar (mult+add, f32 PSUM -> uint16 SBUF) whose uint16
       result IS the bf16 exp approximation (RNE conversion, validated
       on HW; ~0.2% extra end-to-end error) + reciprocal_approx_fast
       + normalize muls + v evictions.
  Pool issues the OT / output DMAs (cheap dispatch, keeps SP free).

Softmax denominator: VE tiles carry a ones-column (col 64 per head), so
PV accumulates l = sum_k P in out row 64; normalization broadcasts l
across partitions with a K=1 matmul, then reciprocal_approx_fast + mul.
"""

import numpy as np
import ml_dtypes
from contextlib import ExitStack

import concourse.bass as bass
import concourse.mybir as mybir
import concourse.tile as tile
from concourse import bacc
from concourse.bass import ts, ds
from concourse.bass_utils import run_bass_kernel_spmd

BF16 = ml_dtypes.bfloat16
DT = mybir.dt.bfloat16
F32 = mybir.dt.float32
U16 = mybir.dt.uint16
EXP = mybir.ActivationFunctionType.Exp

LOG2E = 1.4426950408889634
SCALE = 0.125                    # hd**-0.5
A16 = 128.0 * LOG2E * SCALE      # schraudolph slope (raw-score units)
B16 = 127.0 * 128.0 - 7.5        # schraudolph bias (optimal shift -7.5)

_LAST_RESULTS = None


def build_nc(N=4096, C=512, HD=64):
    KT = N // 128     # 32 key tiles
    QC = N // 512     # 8 query chunks
    CC = C // 128     # 4 channel chunks
    NTQ = 4           # output 128-row tiles per query chunk

    nc = bacc.Bacc("TRN2", target_bir_lowering=False, debug=False)

    xt = nc.dram_tensor("xt", [C, N], DT, kind="ExternalInput").ap()
    wt = nc.dram_tensor("wt", [C, 3 * 128], DT, kind="ExternalInput").ap()
    wpt = nc.dram_tensor("wpt", [128, C], DT, kind="ExternalInput").ap()
    outp = nc.dram_tensor("outp", [N, C], F32, kind="ExternalOutput").ap()

    with tile.TileContext(nc) as tc, ExitStack() as ctx:
        const = ctx.enter_context(tc.tile_pool(name="const", bufs=1))

        X = [const.tile([128, N], DT, tag=f"X{i}", name=f"X{i}") for i in range(CC)]
        W = [const.tile([128, 3 * 128], DT, tag=f"W{i}", name=f"W{i}") for i in range(CC)]
        WPT = const.tile([128, C], DT, tag="WPT", name="WPT")
        Qp = const.tile([128, N], DT, tag="Qp", name="Qp")
        Kp = const.tile([128, N], DT, tag="Kp", name="Kp")
        VE2 = const.tile([128, KT, 2, HD + 1], DT, tag="VE2", name="VE2")
        VE = [VE2[:, i] for i in range(KT)]
        OT = const.tile([128, N], DT, tag="OT", name="OT")
        ones = const.tile([128, 64], F32, tag="ones")
        WPB = const.tile([64, C], DT, tag="WPB", name="WPB")

        nc.vector.memset(ones[:, :], 1.0)

        # ---- input DMAs, split across the SP and Pool issue queues so the
        # critical first fills (W qk + x cols 0:512) land fast; first x
        # chunks are partition-split to engage two DMA engines each.
        for cc in range(CC):
            q = nc.sync if cc < 2 else nc.scalar
            q.dma_start(X[cc][0:64, 0:512], xt[ds(128 * cc, 64), 0:512])
            q.dma_start(X[cc][64:128, 0:512], xt[ds(128 * cc + 64, 64), 0:512])
            q.dma_start(W[cc][:, 0:256], wt[ts(cc, 128), 0:256])
        for cc in range(CC):
            nc.gpsimd.dma_start(X[cc][:, 512:1024], xt[ts(cc, 128), 512:1024])
        for cc in range(CC):
            nc.gpsimd.dma_start(W[cc][:, 256:384], wt[ts(cc, 128), 256:384])
        for cc in range(CC):
            nc.gpsimd.dma_start(X[cc][:, 1024:1536], xt[ts(cc, 128), 1024:1536])
        for cc in range(CC):
            nc.gpsimd.dma_start(X[cc][:, 1536:2048], xt[ts(cc, 128), 1536:2048])
        for blk in range(2, 4):
            for cc in range(CC):
                nc.gpsimd.dma_start(
                    X[cc][:, ds(1024 * blk, 1024)], xt[ts(cc, 128), ds(1024 * blk, 1024)]
                )
        nc.gpsimd.dma_start(WPT[:], wpt[:, :])
        nc.gpsimd.dma_start(WPB[:], wpt[64:128, :])

        with (
            tc.tile_pool(name="s_ps", bufs=2, space="PSUM") as s_ps,
            tc.tile_pool(name="o_ps", bufs=2, space="PSUM") as o_ps,
            tc.tile_pool(name="m_ps", bufs=2, space="PSUM") as m_ps,
            tc.tile_pool(name="p_sb", bufs=6) as p_sb,
            tc.tile_pool(name="t_sb", bufs=4) as t_sb,
            tc.tile_pool(name="ob_sb", bufs=3) as ob_sb,
        ):
            # ---------------- fill emitters ----------------
            def emit_q(qc2):
                ps = m_ps.tile([128, 512], F32, tag="m", name=f"qf{qc2}")
                for cc in range(CC):
                    nc.tensor.matmul(
                        ps[:], W[cc][:, 0:128], X[cc][:, ts(qc2, 512)],
                        start=(cc == 0), stop=(cc == CC - 1),
                    )
                nc.scalar.copy(Qp[:, ts(qc2, 512)], ps[:])

            def emit_k(kc2):
                ps = m_ps.tile([128, 512], F32, tag="m", name=f"kf{kc2}")
                for cc in range(CC):
                    nc.tensor.matmul(
                        ps[:], W[cc][:, 128:256], X[cc][:, ts(kc2, 512)],
                        start=(cc == 0), stop=(cc == CC - 1),
                    )
                nc.scalar.copy(Kp[:, ts(kc2, 512)], ps[:])

            def emit_v(kt2):
                ps = m_ps.tile([128, 128], F32, tag="m", name=f"vf{kt2}")
                for cc in range(CC):
                    nc.tensor.matmul(
                        ps[:], X[cc][:, ts(kt2, 128)], W[cc][:, 256:384],
                        start=(cc == 0), stop=(cc == CC - 1),
                    )
                nc.vector.memset(VE2[:, kt2, :, HD:HD + 1], 1.0)
                nc.vector.tensor_copy(
                    VE2[:, kt2, :, 0:HD], ps[:].rearrange("p (h d) -> p h d", h=2)
                )

            # ---------------- normalization + projection ----------------
            def make_evicts(ocA, oA, ocB, oB):
                def _e():
                    nc.scalar.copy(ocA[0:65, :], oA[0:65, :])
                    nc.vector.tensor_copy(ocB[0:65, :], oB[0:65, :])
                return _e

            def emit_norm(qc2, half, oc, c0=0, cn=512, to_ot=True):
                rb = m_ps.tile([64, 512], F32, tag="m", name=f"rb{qc2}_{half}_{c0}")
                nc.tensor.matmul(
                    rb[:, 0:cn], ones[64:65, 0:64], oc[64:65, ds(c0, cn)],
                    start=True, stop=True,
                )
                rlb = t_sb.tile([64, 512], F32, tag="rlb", name=f"rlb{qc2}_{half}_{c0}")
                nc.vector.reciprocal_approx_fast(rlb[:, 0:cn], rb[:, 0:cn])
                tb = t_sb.tile([64, 512], DT, tag="tb", name=f"tb{qc2}_{half}_{c0}")
                nc.vector.tensor_mul(tb[:, 0:cn], oc[0:64, ds(c0, cn)], rlb[:, 0:cn])
                if to_ot:
                    nc.gpsimd.dma_start(
                        OT[ds(64 * half, 64), ds(qc2 * 512 + c0, cn)], tb[:, 0:cn]
                    )
                return tb

            def make_norm(qc2, half, oc, c0=0, cn=512):
                def _n():
                    emit_norm(qc2, half, oc, c0, cn)
                return _n

            def make_proj(nt):
                def _p():
                    pf = m_ps.tile([128, 512], F32, tag="m", name=f"pf{nt}")
                    nc.tensor.matmul(pf[:], OT[:, ts(nt, 128)], WPT[:], start=True, stop=True)
                    ob = ob_sb.tile([128, C], F32, tag="ob", name=f"ob{nt}")
                    nc.scalar.copy(ob[:], pf[:])
                    nc.sync.dma_start(outp[ds(128 * nt, 64), :], ob[0:64, :])
                    nc.sync.dma_start(outp[ds(128 * nt + 64, 64), :], ob[64:128, :])
                return _p

            pending = []

            def finalize(qc2, oA, oB):
                if qc2 != QC - 1:
                    ocA = t_sb.tile([128, 512], F32, tag="oc", name=f"ocA{qc2}")
                    ocB = t_sb.tile([128, 512], F32, tag="ocb", name=f"ocB{qc2}")
                    pending.append(make_evicts(ocA, oA, ocB, oB))
                if qc2 == QC - 1:
                    # tail: 128-col pieces with per-piece PSUM evictions;
                    # proj reads the normalized tb tiles directly (two K=64
                    # matmuls); output DMAs rotate across all issue queues
                    dmaq = (nc.sync, nc.scalar, nc.gpsimd, nc.sync)

                    def make_tail_piece(i, oA=oA, oB=oB):
                        def _t():
                            ocA = t_sb.tile([128, 128], F32, tag="occ", name=f"tocA{i}")
                            ocB = t_sb.tile([128, 128], F32, tag="occ", name=f"tocB{i}")
                            nc.scalar.copy(ocA[0:65, :], oA[0:65, ts(i, 128)])
                            nc.vector.tensor_copy(ocB[0:65, :], oB[0:65, ts(i, 128)])
                            tbA = emit_norm(qc2, 0, ocA, 0, 128, to_ot=False)
                            tbB = emit_norm(qc2, 1, ocB, 0, 128, to_ot=False)
                            nt = qc2 * NTQ + i
                            pf = m_ps.tile([128, 512], F32, tag="m", name=f"pf{nt}")
                            nc.tensor.matmul(
                                pf[:], tbA[:, 0:128], WPT[0:64, :],
                                start=True, stop=False,
                            )
                            nc.tensor.matmul(
                                pf[:], tbB[:, 0:128], WPB[:],
                                start=False, stop=True,
                            )
                            ob = ob_sb.tile([128, C], F32, tag="ob", name=f"ob{nt}")
                            nc.scalar.copy(ob[:], pf[:])
                            for qtr in range(4):
                                dmaq[qtr].dma_start(
                                    outp[ds(128 * nt + 32 * qtr, 32), :],
                                    ob[ds(32 * qtr, 32), :],
                                )
                        return _t
                    for i in range(NTQ):
                        pending.append(make_tail_piece(i))
                else:
                    pending.append(make_norm(qc2, 0, ocA))
                    pending.append(make_norm(qc2, 1, ocB))
                    for i in range(NTQ):
                        pending.append(make_proj(qc2 * NTQ + i))

            o_tiles = {}

            def emit_pv(pe):
                pp, pkt, pqc = pe
                if pkt == 0:
                    o_tiles[pqc] = (
                        o_ps.tile([128, 512], F32, tag="o", name=f"oA{pqc}"),
                        o_ps.tile([128, 512], F32, tag="o", name=f"oB{pqc}"),
                    )
                poA, poB = o_tiles[pqc]
                last = pkt == KT - 1
                nc.tensor.matmul(
                    poA[0:65, :], VE2[:, pkt, 0, :], pp[:, 0:512],
                    start=(pkt == 0), stop=last,
                )
                nc.tensor.matmul(
                    poB[0:65, :], VE2[:, pkt, 1, :], pp[:, 512:1024],
                    start=(pkt == 0), stop=last,
                )
                if last:
                    finalize(pqc, poA, poB)

            # ---------------- fill scheduling ----------------
            fill_jobs = []
            for kt in range(8, KT, 2):
                fill_jobs.append((max(0, kt - 4), ("v", kt)))
            for kc in range(2, QC):
                fill_jobs.append((max(0, 4 * kc - 4), ("k", kc)))
            for qc in range(1, QC):
                fill_jobs.append((max(0, KT * qc - 8), ("q", qc)))
            fill_jobs.sort(key=lambda j: j[0])

            def do_fill(job):
                kind = job[0]
                if kind == "v":
                    emit_v(job[1])
                    emit_v(job[1] + 1)
                elif kind == "k":
                    emit_k(job[1])
                else:
                    emit_q(job[1])

            # upfront fills: everything x cols 0:1024 can feed (fills the
            # input-DMA wait time with PE work)
            emit_q(0)
            emit_k(0)
            emit_k(1)
            for kt in range(8):
                emit_v(kt)

            # exp engine pattern: 4 ScalarE : 3 DVE per 7 steps
            PAT = ("s", "d", "s", "d", "s", "d", "s")

            def emit_exp(p, s, pos):
                if PAT[pos % 7] == "s":
                    nc.scalar.activation(p[:], s[:], EXP, scale=SCALE)
                else:
                    nc.vector.tensor_scalar(
                        p[:].bitcast(U16), s[:], A16, B16,
                        mybir.AluOpType.mult, mybir.AluOpType.add,
                    )

            # ---------------- main loop ----------------
            # two steps per group: S-pair, S-pair (same array geometry,
            # pipelined), both exps, then four PV matmuls — one K-width
            # switch per direction per group instead of two
            pendq = []
            for qc in range(QC):
                for kt2 in range(0, KT, 2):
                    pos = qc * KT + kt2
                    while fill_jobs and fill_jobs[0][0] <= pos:
                        do_fill(fill_jobs.pop(0)[1])
                    if pending:
                        pending.pop(0)()
                    group = []
                    for kt in (kt2, kt2 + 1):
                        s = s_ps.tile([128, 1024], F32, tag="s", name=f"s{qc}_{kt}")
                        nc.tensor.matmul(
                            s[:, 0:512], Kp[0:64, ts(kt, 128)], Qp[0:64, ts(qc, 512)],
                            start=True, stop=True,
                        )
                        nc.tensor.matmul(
                            s[:, 512:1024], Kp[64:128, ts(kt, 128)], Qp[64:128, ts(qc, 512)],
                            start=True, stop=True,
                        )
                        group.append((s, kt))
                    for i, (s, kt) in enumerate(group):
                        p = p_sb.tile([128, 1024], DT, tag="p", name=f"p{qc}_{kt}")
                        emit_exp(p, s, pos + i)
                        pendq.append((p, kt, qc))
                    if len(pendq) == 6:
                        emit_pv(pendq.pop(0))
                        emit_pv(pendq.pop(0))
            while pendq:
                emit_pv(pendq.pop(0))
            while pending:
                pending.pop(0)()

    nc.compile()
    return nc


_NC_CACHE = {}


def _get_nc(key=(4096, 512, 64)):
    if key not in _NC_CACHE:
        _NC_CACHE[key] = build_nc(*key)
    return _NC_CACHE[key]


def make_in_maps(x, w_qkv, w_proj):
    B = x.shape[0]
    xtb = [np.ascontiguousarray(x[b].T).astype(BF16) for b in range(B)]
    in_maps = []
    for c in range(8):
        b, hp = c // 4, c % 4
        sl = slice(128 * hp, 128 * (hp + 1))
        wt = np.concatenate(
            [
                w_qkv[0:512][sl].T,
                w_qkv[512:1024][sl].T,
                w_qkv[1024:1536][sl].T,
            ],
            axis=1,
        )
        in_maps.append(
            {
                "xt": xtb[b],
                "wt": np.ascontiguousarray(wt).astype(BF16),
                "wpt": np.ascontiguousarray(w_proj[:, sl].T).astype(BF16),
            }
        )
    return in_maps


def kernel(x, w_qkv, w_proj, b_proj):
    x = np.asarray(x, dtype=np.float32)
    w_qkv = np.asarray(w_qkv, dtype=np.float32)
    w_proj = np.asarray(w_proj, dtype=np.float32)
    b_proj = np.asarray(b_proj, dtype=np.float32)
    nc = _get_nc()
    in_maps = make_in_maps(x, w_qkv, w_proj)
    res = run_bass_kernel_spmd(nc, in_maps, core_ids=list(range(8)))
    global _LAST_RESULTS
    _LAST_RESULTS = res
    B, N, C = x.shape
    out = np.empty((B, N, C), np.float32)
    for b in range(B):
        acc = res.results[4 * b]["outp"].astype(np.float32).copy()
        for hp in range(1, 4):
            acc += res.results[4 * b + hp]["outp"]
        out[b] = acc + b_proj[None, :]
    return out


# revision 25
# speedup vs baseline: 1.0196x; 1.0196x over previous
"""Distributed flash-attention Bass kernel for 8 TRN2 NeuronCores.

Problem: nn_Attention (B=2, N=4096, C=512, H=8 heads, hd=64), f32 I/O.

Sharding: batch x head-pair. Core c handles batch c//4, heads {2*(c%4),
2*(c%4)+1}, over the FULL 4096-query x 4096-key attention. Each core
projects q/k/v for only its two heads (w_qkv column slice -> no
duplicated K/V compute), runs no-max flash attention, and applies its
w_proj slice to produce a PARTIAL [4096, 512] f32 output. The host sums
the 4 partials per batch and adds b_proj (numpy; no device collectives).

Engine plan (measured steady-state rates):
  PE   per step (128 keys x 2 heads x 512 queries): S-pair = two
       row-tiled K=64 matmuls running concurrently in PE row groups
       (~385ns), then two PV K=128 matmuls (~215ns each, drains
       overlapped). Fills/norm-broadcast/proj matmuls are paced into
       the stream. PV trails S/exp by TWO steps so the in-order PE
       queue never waits on the exp latency.
  Act  exp of 4/7 of the [128,1024] score tiles (exact table exp,
       whole tiles to amortize overhead) + PSUM evictions.
  DVE  exp of 3/7 of tiles via the Schraudolph bit-trick: a single
       tensor_scalar (mult+add, f32 PSUM -> uint16 SBUF) whose uint16
       result IS the bf16 exp approximation (RNE conversion, validated
       on HW; ~0.2% extra end-to-end error) + reciprocal_approx_fast
       + normalize muls + v evictions.
  Pool issues the OT / output DMAs (cheap dispatch, keeps SP free).

Softmax denominator: VE tiles carry a ones-column (col 64 per head), so
PV accumulates l = sum_k P in out row 64; normalization broadcasts l
across partitions with a K=1 matmul, then reciprocal_approx_fast + mul.
"""

import numpy as np
import ml_dtypes
from contextlib import ExitStack

import concourse.bass as bass
import concourse.mybir as mybir
import concourse.tile as tile
from concourse import bacc
from concourse.bass import ts, ds
from concourse.bass_utils import run_bass_kernel_spmd

BF16 = ml_dtypes.bfloat16
DT = mybir.dt.bfloat16
F32 = mybir.dt.float32
U16 = mybir.dt.uint16
EXP = mybir.ActivationFunctionType.Exp

LOG2E = 1.4426950408889634
SCALE = 0.125                    # hd**-0.5
A16 = 128.0 * LOG2E * SCALE      # schraudolph slope (raw-score units)
B16 = 127.0 * 128.0 - 7.5        # schraudolph bias (optimal shift -7.5)

_LAST_RESULTS = None


def build_nc(N=4096, C=512, HD=64):
    KT = N // 128     # 32 key tiles
    QC = N // 512     # 8 query chunks
    CC = C // 128     # 4 channel chunks
    NTQ = 4           # output 128-row tiles per query chunk

    nc = bacc.Bacc("TRN2", target_bir_lowering=False, debug=False)

    xt = nc.dram_tensor("xt", [C, N], DT, kind="ExternalInput").ap()
    wt = nc.dram_tensor("wt", [C, 3 * 128], DT, kind="ExternalInput").ap()
    wpt = nc.dram_tensor("wpt", [128, C], DT, kind="ExternalInput").ap()
    outp = nc.dram_tensor("outp", [N, C], F32, kind="ExternalOutput").ap()

    with tile.TileContext(nc) as tc, ExitStack() as ctx:
        const = ctx.enter_context(tc.tile_pool(name="const", bufs=1))

        X = [const.tile([128, N], DT, tag=f"X{i}", name=f"X{i}") for i in range(CC)]
        W = [const.tile([128, 3 * 128], DT, tag=f"W{i}", name=f"W{i}") for i in range(CC)]
        WPT = const.tile([128, C], DT, tag="WPT", name="WPT")
        Qp = const.tile([128, N], DT, tag="Qp", name="Qp")
        Kp = const.tile([128, N], DT, tag="Kp", name="Kp")
        VE2 = const.tile([128, KT, 2, HD + 1], DT, tag="VE2", name="VE2")
        VE = [VE2[:, i] for i in range(KT)]
        OT = const.tile([128, N], DT, tag="OT", name="OT")
        ones = const.tile([128, 64], F32, tag="ones")
        WPB = const.tile([64, C], DT, tag="WPB", name="WPB")

        nc.vector.memset(ones[:, :], 1.0)

        # ---- input DMAs, split across the SP and Pool issue queues so the
        # critical first fills (W qk + x cols 0:512) land fast; first x
        # chunks are partition-split to engage two DMA engines each.
        qs = (nc.sync, nc.scalar, nc.gpsimd, nc.sync)
        for cc in range(CC):
            q = qs[cc]
            q.dma_start(X[cc][0:64, 0:512], xt[ds(128 * cc, 64), 0:512])
            q.dma_start(X[cc][64:128, 0:512], xt[ds(128 * cc + 64, 64), 0:512])
            q.dma_start(W[cc][:, 0:256], wt[ts(cc, 128), 0:256])
        for cc in range(CC):
            nc.gpsimd.dma_start(X[cc][:, 512:1024], xt[ts(cc, 128), 512:1024])
        for cc in range(CC):
            nc.gpsimd.dma_start(W[cc][:, 256:384], wt[ts(cc, 128), 256:384])
        for cc in range(CC):
            nc.gpsimd.dma_start(X[cc][:, 1024:1536], xt[ts(cc, 128), 1024:1536])
        for cc in range(CC):
            nc.gpsimd.dma_start(X[cc][:, 1536:2048], xt[ts(cc, 128), 1536:2048])
        for blk in range(2, 4):
            for cc in range(CC):
                nc.gpsimd.dma_start(
                    X[cc][:, ds(1024 * blk, 1024)], xt[ts(cc, 128), ds(1024 * blk, 1024)]
                )
        nc.gpsimd.dma_start(WPT[:], wpt[:, :])
        nc.gpsimd.dma_start(WPB[:], wpt[64:128, :])

        with (
            tc.tile_pool(name="s_ps", bufs=2, space="PSUM") as s_ps,
            tc.tile_pool(name="o_ps", bufs=2, space="PSUM") as o_ps,
            tc.tile_pool(name="m_ps", bufs=2, space="PSUM") as m_ps,
            tc.tile_pool(name="p_sb", bufs=6) as p_sb,
            tc.tile_pool(name="t_sb", bufs=4) as t_sb,
            tc.tile_pool(name="ob_sb", bufs=3) as ob_sb,
        ):
            # ---------------- fill emitters ----------------
            def emit_q(qc2):
                ps = m_ps.tile([128, 512], F32, tag="m", name=f"qf{qc2}")
                for cc in range(CC):
                    nc.tensor.matmul(
                        ps[:], W[cc][:, 0:128], X[cc][:, ts(qc2, 512)],
                        start=(cc == 0), stop=(cc == CC - 1),
                    )
                nc.scalar.copy(Qp[:, ts(qc2, 512)], ps[:])

            def emit_k(kc2):
                ps = m_ps.tile([128, 512], F32, tag="m", name=f"kf{kc2}")
                for cc in range(CC):
                    nc.tensor.matmul(
                        ps[:], W[cc][:, 128:256], X[cc][:, ts(kc2, 512)],
                        start=(cc == 0), stop=(cc == CC - 1),
                    )
                nc.scalar.copy(Kp[:, ts(kc2, 512)], ps[:])

            def emit_v(kt2):
                ps = m_ps.tile([128, 128], F32, tag="m", name=f"vf{kt2}")
                for cc in range(CC):
                    nc.tensor.matmul(
                        ps[:], X[cc][:, ts(kt2, 128)], W[cc][:, 256:384],
                        start=(cc == 0), stop=(cc == CC - 1),
                    )
                nc.vector.memset(VE2[:, kt2, :, HD:HD + 1], 1.0)
                nc.vector.tensor_copy(
                    VE2[:, kt2, :, 0:HD], ps[:].rearrange("p (h d) -> p h d", h=2)
                )

            # ---------------- normalization + projection ----------------
            def make_evicts(ocA, oA, ocB, oB):
                def _e():
                    nc.scalar.copy(ocA[0:65, :], oA[0:65, :])
                    nc.vector.tensor_copy(ocB[0:65, :], oB[0:65, :])
                return _e

            def emit_norm(qc2, half, oc, c0=0, cn=512, to_ot=True):
                rb = m_ps.tile([64, 512], F32, tag="m", name=f"rb{qc2}_{half}_{c0}")
                nc.tensor.matmul(
                    rb[:, 0:cn], ones[64:65, 0:64], oc[64:65, ds(c0, cn)],
                    start=True, stop=True,
                )
                rlb = t_sb.tile([64, 512], F32, tag="rlb", name=f"rlb{qc2}_{half}_{c0}")
                nc.vector.reciprocal_approx_fast(rlb[:, 0:cn], rb[:, 0:cn])
                tb = t_sb.tile([64, 512], DT, tag="tb", name=f"tb{qc2}_{half}_{c0}")
                nc.vector.tensor_mul(tb[:, 0:cn], oc[0:64, ds(c0, cn)], rlb[:, 0:cn])
                if to_ot:
                    nc.gpsimd.dma_start(
                        OT[ds(64 * half, 64), ds(qc2 * 512 + c0, cn)], tb[:, 0:cn]
                    )
                return tb

            def make_norm(qc2, half, oc, c0=0, cn=512):
                def _n():
                    emit_norm(qc2, half, oc, c0, cn)
                return _n

            def make_proj(nt):
                def _p():
                    pf = m_ps.tile([128, 512], F32, tag="m", name=f"pf{nt}")
                    nc.tensor.matmul(pf[:], OT[:, ts(nt, 128)], WPT[:], start=True, stop=True)
                    ob = ob_sb.tile([128, C], F32, tag="ob", name=f"ob{nt}")
                    nc.scalar.copy(ob[:], pf[:])
                    nc.sync.dma_start(outp[ds(128 * nt, 64), :], ob[0:64, :])
                    nc.sync.dma_start(outp[ds(128 * nt + 64, 64), :], ob[64:128, :])
                return _p

            pending = []

            def finalize(qc2, oA, oB):
                if qc2 != QC - 1:
                    ocA = t_sb.tile([128, 512], F32, tag="oc", name=f"ocA{qc2}")
                    ocB = t_sb.tile([128, 512], F32, tag="ocb", name=f"ocB{qc2}")
                    pending.append(make_evicts(ocA, oA, ocB, oB))
                if qc2 == QC - 1:
                    # tail: 128-col pieces with per-piece PSUM evictions;
                    # proj reads the normalized tb tiles directly (two K=64
                    # matmuls); output DMAs rotate across all issue queues
                    dmaq = (nc.sync, nc.scalar, nc.gpsimd, nc.sync)

                    def make_tail_piece(i, oA=oA, oB=oB):
                        def _t():
                            ocA = t_sb.tile([128, 128], F32, tag="occ", name=f"tocA{i}")
                            ocB = t_sb.tile([128, 128], F32, tag="occ", name=f"tocB{i}")
                            nc.scalar.copy(ocA[0:65, :], oA[0:65, ts(i, 128)])
                            nc.vector.tensor_copy(ocB[0:65, :], oB[0:65, ts(i, 128)])
                            tbA = emit_norm(qc2, 0, ocA, 0, 128, to_ot=False)
                            tbB = emit_norm(qc2, 1, ocB, 0, 128, to_ot=False)
                            nt = qc2 * NTQ + i
                            pf = m_ps.tile([128, 512], F32, tag="m", name=f"pf{nt}")
                            nc.tensor.matmul(
                                pf[:], tbA[:, 0:128], WPT[0:64, :],
                                start=True, stop=False,
                            )
                            nc.tensor.matmul(
                                pf[:], tbB[:, 0:128], WPB[:],
                                start=False, stop=True,
                            )
                            ob = ob_sb.tile([128, C], F32, tag="ob", name=f"ob{nt}")
                            nc.scalar.copy(ob[:], pf[:])
                            for qtr in range(4):
                                dmaq[qtr].dma_start(
                                    outp[ds(128 * nt + 32 * qtr, 32), :],
                                    ob[ds(32 * qtr, 32), :],
                                )
                        return _t
                    for i in range(NTQ):
                        pending.append(make_tail_piece(i))
                else:
                    pending.append(make_norm(qc2, 0, ocA))
                    pending.append(make_norm(qc2, 1, ocB))
                    for i in range(NTQ):
                        pending.append(make_proj(qc2 * NTQ + i))

            o_tiles = {}

            def emit_pv(pe):
                pp, pkt, pqc = pe
                if pkt == 0:
                    o_tiles[pqc] = (
                        o_ps.tile([128, 512], F32, tag="o", name=f"oA{pqc}"),
                        o_ps.tile([128, 512], F32, tag="o", name=f"oB{pqc}"),
                    )
                poA, poB = o_tiles[pqc]
                last = pkt == KT - 1
                nc.tensor.matmul(
                    poA[0:65, :], VE2[:, pkt, 0, :], pp[:, 0:512],
                    start=(pkt == 0), stop=last,
                )
                nc.tensor.matmul(
                    poB[0:65, :], VE2[:, pkt, 1, :], pp[:, 512:1024],
                    start=(pkt == 0), stop=last,
                )
                if last:
                    finalize(pqc, poA, poB)

            # ---------------- fill scheduling ----------------
            fill_jobs = []
            for kt in range(8, KT, 2):
                fill_jobs.append((max(0, kt - 4), ("v", kt)))
            for kc in range(2, QC):
                fill_jobs.append((max(0, 4 * kc - 4), ("k", kc)))
            for qc in range(1, QC):
                fill_jobs.append((max(0, KT * qc - 8), ("q", qc)))
            fill_jobs.sort(key=lambda j: j[0])

            def do_fill(job):
                kind = job[0]
                if kind == "v":
                    emit_v(job[1])
                    emit_v(job[1] + 1)
                elif kind == "k":
                    emit_k(job[1])
                else:
                    emit_q(job[1])

            # upfront fills: everything x cols 0:1024 can feed (fills the
            # input-DMA wait time with PE work)
            emit_q(0)
            emit_k(0)
            emit_k(1)
            for kt in range(8):
                emit_v(kt)

            # exp engine pattern: 4 ScalarE : 3 DVE per 7 steps
            PAT = ("s", "d", "s", "d", "s", "d", "s")

            def emit_exp(p, s, pos):
                if PAT[pos % 7] == "s":
                    nc.scalar.activation(p[:], s[:], EXP, scale=SCALE)
                else:
                    nc.vector.tensor_scalar(
                        p[:].bitcast(U16), s[:], A16, B16,
                        mybir.AluOpType.mult, mybir.AluOpType.add,
                    )

            # ---------------- main loop ----------------
            # two steps per group: S-pair, S-pair (same array geometry,
            # pipelined), both exps, then four PV matmuls — one K-width
            # switch per direction per group instead of two
            pendq = []
            for qc in range(QC):
                for kt2 in range(0, KT, 2):
                    pos = qc * KT + kt2
                    while fill_jobs and fill_jobs[0][0] <= pos:
                        do_fill(fill_jobs.pop(0)[1])
                    if pending:
                        pending.pop(0)()
                    group = []
                    for kt in (kt2, kt2 + 1):
                        s = s_ps.tile([128, 1024], F32, tag="s", name=f"s{qc}_{kt}")
                        nc.tensor.matmul(
                            s[:, 0:512], Kp[0:64, ts(kt, 128)], Qp[0:64, ts(qc, 512)],
                            start=True, stop=True,
                        )
                        nc.tensor.matmul(
                            s[:, 512:1024], Kp[64:128, ts(kt, 128)], Qp[64:128, ts(qc, 512)],
                            start=True, stop=True,
                        )
                        group.append((s, kt))
                    for i, (s, kt) in enumerate(group):
                        p = p_sb.tile([128, 1024], DT, tag="p", name=f"p{qc}_{kt}")
                        emit_exp(p, s, pos + i)
                        pendq.append((p, kt, qc))
                    if len(pendq) == 6:
                        emit_pv(pendq.pop(0))
                        emit_pv(pendq.pop(0))
            while pendq:
                emit_pv(pendq.pop(0))
            while pending:
                pending.pop(0)()

    nc.compile()
    return nc


_NC_CACHE = {}


def _get_nc(key=(4096, 512, 64)):
    if key not in _NC_CACHE:
        _NC_CACHE[key] = build_nc(*key)
    return _NC_CACHE[key]


def make_in_maps(x, w_qkv, w_proj):
    B = x.shape[0]
    xtb = [np.ascontiguousarray(x[b].T).astype(BF16) for b in range(B)]
    in_maps = []
    for c in range(8):
        b, hp = c // 4, c % 4
        sl = slice(128 * hp, 128 * (hp + 1))
        wt = np.concatenate(
            [
                w_qkv[0:512][sl].T,
                w_qkv[512:1024][sl].T,
                w_qkv[1024:1536][sl].T,
            ],
            axis=1,
        )
        in_maps.append(
            {
                "xt": xtb[b],
                "wt": np.ascontiguousarray(wt).astype(BF16),
                "wpt": np.ascontiguousarray(w_proj[:, sl].T).astype(BF16),
            }
        )
    return in_maps


def kernel(x, w_qkv, w_proj, b_proj):
    x = np.asarray(x, dtype=np.float32)
    w_qkv = np.asarray(w_qkv, dtype=np.float32)
    w_proj = np.asarray(w_proj, dtype=np.float32)
    b_proj = np.asarray(b_proj, dtype=np.float32)
    nc = _get_nc()
    in_maps = make_in_maps(x, w_qkv, w_proj)
    res = run_bass_kernel_spmd(nc, in_maps, core_ids=list(range(8)))
    global _LAST_RESULTS
    _LAST_RESULTS = res
    B, N, C = x.shape
    out = np.empty((B, N, C), np.float32)
    for b in range(B):
        acc = res.results[4 * b]["outp"].astype(np.float32).copy()
        for hp in range(1, 4):
            acc += res.results[4 * b + hp]["outp"]
        out[b] = acc + b_proj[None, :]
    return out
